# revision 1
# baseline (speedup 1.0000x reference)
"""Trainium2 Bass kernel for MHA with RoPE (dense transformer block).

Problem shapes: h [1, 4096, 1024], 16 heads x 64 dim, full (non-causal)
softmax attention, post-softmax all-ones mask (identity), torch-Linear
projections with bias.

Sharding: head-parallel across 8 cores (2 heads/core). Each core:
  - reads full hT (pre-transposed on host, [1024, 4096])
  - computes qT/kT/vT for its 2 heads (feature-major [128, S])
  - RoPE on qT/kT (sign-folded sin table, partition-crossed muls)
  - per 1024-query chunk: scoresT = kT-tiles x qT (MM_DT matmuls),
    exp on ACT (scale=1/8 fused, no max subtraction: |scores| <= ~7),
    PV with a ones-row appended to v (denominator for free),
    normalize with reciprocal broadcast via K=1 matmul,
  - o_proj partial [S, 1024] with its 128 wo columns.
Host sums the 8 partials and adds bo.

All matmul operands are MM_DT (float32r by default — full-rate on the PE
at N>=256; bf16 fallback). The BIR verifier requires fp32r operands to be
produced rounded, so every matmul input tile is MM_DT-typed and written
by a casting producer (gpsimd DMA-cast load, DVE out-cast, ACT out-cast).
"""

import numpy as np

HIDDEN = 1024
HEADS = 16
HEAD_DIM = 64
SEQ = 4096
NCORES = 8
FPC = 128  # features per core = 2 heads x 64

_NC_CACHE = {}


def _build_nc(S=SEQ, mm_dt="float32r"):
    import concourse.bass as bass
    import concourse.tile as tile
    from concourse import mybir
    from concourse.masks import make_identity
    from contextlib import ExitStack

    f32 = mybir.dt.float32
    MM = getattr(mybir.dt, mm_dt)
    Exp = mybir.ActivationFunctionType.Exp

    D = HEAD_DIM
    HID = HIDDEN
    KT = HID // 128          # hidden contraction tiles
    PC = 512                 # projection seq chunk
    NPC = S // PC
    CH = min(1024, S)        # attention query chunk
    HF = CH // 2             # psum half-chunk
    NCH = S // CH
    SK = S // 128            # key tiles

    nc = bass.Bass(trn_type="TRN2")

    # matmul-facing inputs are pre-converted to MM dtype on the host so the
    # loads go over fast HWDGE queues with no cast
    hT = nc.dram_tensor("hT", [HID, S], MM, kind="ExternalInput")
    wqT = nc.dram_tensor("wqT", [HID, FPC], MM, kind="ExternalInput")
    wkT = nc.dram_tensor("wkT", [HID, FPC], MM, kind="ExternalInput")
    wvT = nc.dram_tensor("wvT", [HID, FPC], MM, kind="ExternalInput")
    bqkv = nc.dram_tensor("bqkv", [FPC, 3], f32, kind="ExternalInput")
    woT = nc.dram_tensor("woT", [FPC, HID], MM, kind="ExternalInput")
    cosT = nc.dram_tensor("cosT", [D, S], f32, kind="ExternalInput")
    sinTs = nc.dram_tensor("sinTs", [D, S], f32, kind="ExternalInput")
    out = nc.dram_tensor("out", [S, HID], f32, kind="ExternalOutput")

    hT3 = hT[:, :].rearrange("(ko p) s -> p ko s", p=128)

    with tile.TileContext(nc) as tc, ExitStack() as top:
        sing = top.enter_context(tc.tile_pool(name="sing", bufs=1))

        wq_sb = sing.tile([128, KT, FPC], MM)
        wk_sb = sing.tile([128, KT, FPC], MM)
        wv_sb = sing.tile([128, KT, FPC], MM)
        nc.sync.dma_start(wq_sb, wqT[:, :].rearrange("(ko p) f -> p ko f", p=128))
        nc.sync.dma_start(wk_sb, wkT[:, :].rearrange("(ko p) f -> p ko f", p=128))
        nc.sync.dma_start(wv_sb, wvT[:, :].rearrange("(ko p) f -> p ko f", p=128))
        wo_sb = sing.tile([FPC, HID], MM)
        nc.sync.dma_start(wo_sb, woT[:, :])
        b_sb = sing.tile([FPC, 3], f32)
        nc.sync.dma_start(b_sb, bqkv[:, :])
        cos_sb = sing.tile([128, S], f32)
        sin_sb = sing.tile([128, S], f32)
        nc.sync.dma_start(cos_sb[0:64, :], cosT[:, :])
        nc.sync.dma_start(cos_sb[64:128, :], cosT[:, :])
        nc.sync.dma_start(sin_sb[0:64, :], sinTs[:, :])
        nc.sync.dma_start(sin_sb[64:128, :], sinTs[:, :])
        ones_sb = sing.tile([1, 64], f32)
        nc.vector.memset(ones_sb, 1.0)
        ident = sing.tile([128, 128], f32)
        make_identity(nc, ident)

        qT_sb = sing.tile([128, S], MM)
        # k is kept as two zero-padded copies (head A in rows 0-63 of kpA,
        # head B in rows 64-127 of kpB, other rows zero): QK then contracts
        # K=128 against the full stacked q tile — the zero rows annihilate the
        # cross-head terms exactly, and the full-128 weight block qualifies
        # for fast weight load / keeps the PE array fully occupied.
        kpA_sb = sing.tile([128, S], MM)
        kpB_sb = sing.tile([128, S], MM)
        nc.vector.memset(kpA_sb, 0.0)
        nc.vector.memset(kpB_sb, 0.0)
        v1_sb = sing.tile([128, 2, SK, 65], MM)
        if mm_dt == "float32r":
            # memset can't write f32r: DMA-broadcast the ones column (the
            # denominator row of the PV matmul) from an inline constant
            ones_dram = nc.inline_tensor(
                np.ones((128, 1), dtype=np.float32), name="onecol"
            )
            ones_bcast = bass.AP(
                tensor=ones_dram,
                offset=0,
                ap=[[1, 128], [0, 2 * SK], [1, 1]],
            )
            v1_flat = v1_sb.rearrange("p a b c -> p (a b) c")
            nc.gpsimd.dma_start(v1_flat[:, :, 64:65], ones_bcast)
        else:
            nc.vector.memset(v1_sb, 1.0)
        ctx_sb = sing.tile([128, S], MM)

        # ---- projections + RoPE + v transpose ----
        with ExitStack() as ph1:
            hp = ph1.enter_context(tc.tile_pool(name="hp", bufs=8))
            vt = ph1.enter_context(tc.tile_pool(name="vt", bufs=2))
            rp = ph1.enter_context(tc.tile_pool(name="rope", bufs=4))
            pps = ph1.enter_context(tc.tile_pool(name="pps", bufs=4, space="PSUM"))
            tps = ph1.enter_context(tc.tile_pool(name="tps", bufs=2, space="PSUM"))
            for ch in range(NPC):
                ssl = slice(ch * PC, (ch + 1) * PC)
                h_sb = hp.tile([128, KT, PC], MM)
                nc.sync.dma_start(h_sb, hT3[:, :, ssl])
                for wi, (w_sb, dst) in enumerate(
                    [(wq_sb, qT_sb), (wk_sb, "kpad"), (wv_sb, None)]
                ):
                    ps = pps.tile([128, PC], f32)
                    for k in range(KT):
                        nc.tensor.matmul(
                            ps,
                            w_sb[:, k, :],
                            h_sb[:, k, :],
                            start=(k == 0),
                            stop=(k == KT - 1),
                        )
                    if dst is not None:
                        # bias add on the (idle-in-this-phase) ACT engine ->
                        # f32 staging, then RoPE below writes the MM-typed dst
                        stg = rp.tile([128, PC], f32, tag="stg", name=f"stg_{ch}_{wi}")
                        nc.scalar.activation(
                            stg,
                            ps,
                            mybir.ActivationFunctionType.Identity,
                            bias=b_sb[:, wi : wi + 1],
                        )
                        tmp = rp.tile([128, PC], f32, tag="tmp", name=f"tmp_{ch}_{wi}")
                        # sin table is permuted+sign-folded on host so that the
                        # factor for destination rows `da` sits at source rows
                        # `sa` (keeps both inputs at the same base partition).
                        # Split across DVE and GpSimd — DVE is the projection
                        # phase's bottleneck engine.
                        for pi, (da, sa) in enumerate(
                            ((0, 32), (32, 0), (64, 96), (96, 64))
                        ):
                            eng = nc.vector if pi % 2 == 0 else nc.gpsimd
                            eng.tensor_mul(
                                tmp[da : da + 32, :],
                                stg[sa : sa + 32, :],
                                sin_sb[sa : sa + 32, ssl],
                            )
                        nc.vector.tensor_mul(stg, stg, cos_sb[:, ssl])
                        # final add casts into the MM-typed q/k tensor
                        if dst == "kpad":
                            nc.vector.tensor_add(
                                kpA_sb[0:64, ssl], stg[0:64, :], tmp[0:64, :]
                            )
                            nc.vector.tensor_add(
                                kpB_sb[64:128, ssl], stg[64:128, :], tmp[64:128, :]
                            )
                        else:
                            nc.vector.tensor_add(dst[:, ssl], stg, tmp)
                    else:
                        vtmp = vt.tile([128, PC], f32)
                        nc.scalar.activation(
                            vtmp,
                            ps,
                            mybir.ActivationFunctionType.Identity,
                            bias=b_sb[:, wi : wi + 1],
                        )
                        for st in range(PC // 128):
                            for hh in range(2):
                                tp = tps.tile([128, 64], f32)
                                nc.tensor.transpose(
                                    tp,
                                    vtmp[hh * 64 : hh * 64 + 64, st * 128 : st * 128 + 128],
                                    ident[hh * 64 : hh * 64 + 64, hh * 64 : hh * 64 + 64],
                                )
                                nc.vector.tensor_copy(
                                    v1_sb[:, hh, ch * (PC // 128) + st, 0:64], tp
                                )

        # ---- attention + o_proj ----
        # Software-pipelined: QK(i+1) is emitted before PV(i) so the PE never
        # sits behind the exp chain; each head's PE-side epilogue (recip
        # broadcast, normalize, and for the second head the chunk's o_proj) is
        # deferred into the NEXT head's stream so transition stalls hide under
        # the running QK/exp pipeline. PV results are staged out of PSUM into
        # SBUF immediately to release the accumulator banks for the next head.
        with ExitStack() as ph2:
            sp = ph2.enter_context(tc.tile_pool(name="sp", bufs=2, space="PSUM"))
            cxp = ph2.enter_context(tc.tile_pool(name="cxp", bufs=2, space="PSUM"))
            msp = ph2.enter_context(tc.tile_pool(name="msp", bufs=2, space="PSUM"))
            ptp = ph2.enter_context(tc.tile_pool(name="ptp", bufs=8))
            mss = ph2.enter_context(tc.tile_pool(name="mss", bufs=2))
            osb = ph2.enter_context(tc.tile_pool(name="osb", bufs=3))

            Ln = mybir.ActivationFunctionType.Ln

            def emit_oproj(c, sqs):
                cs0 = c * CH
                for sq in sqs:
                    r0 = cs0 + sq * 128
                    for nz in range(HID // 512):
                        ops = msp.tile(
                            [128, 512], f32, tag="mm", name=f"op_{c}_{sq}_{nz}"
                        )
                        nc.tensor.matmul(
                            ops,
                            ctx_sb[:, r0 : r0 + 128],
                            wo_sb[:, nz * 512 : (nz + 1) * 512],
                            start=True,
                            stop=True,
                        )
                        ob = osb.tile([128, 512], f32, tag="ob", name=f"ob_{c}_{sq}_{nz}")
                        nc.vector.tensor_copy(ob, ops)
                        nc.sync.dma_start(
                            out[r0 : r0 + 128, nz * 512 : (nz + 1) * 512], ob
                        )

            def emit_warmer(tag, n=12):
                # dense burst of throwaway matmuls: the PE clock gate (HAM)
                # un-throttles only after a sustained-busy window, and the
                # regular QK/exp/PV interleave never looks busy enough.
                # M=1 keeps the per-matmul LDWEIGHTS bubble to ~27ns (the
                # weight load scales with columns) so the burst is ~98% duty;
                # rotate two PSUM tiles so same-bank WAW doesn't serialize.
                wps = [
                    msp.tile([1, 512], f32, tag="mm", name=f"warm_{tag}_{z}")
                    for z in range(2)
                ]
                for j in range(n):
                    nc.tensor.matmul(
                        wps[j % 2],
                        wq_sb[:, j % KT, 0:1],
                        qT_sb[:, 0:512],
                        start=True,
                        stop=True,
                        skip_group_check=True,
                    )

            def emit_head(c, hh, work1, work2):
                """Emit one head's stream; `work1`/`work2` are deferred PE work
                from the previous streams (epilogues / o_proj halves), dropped
                in mid-stream once their ACT/DVE inputs are long since ready."""
                cs0 = c * CH
                hsl = slice(hh * 64, hh * 64 + 64)
                cx = [
                    cxp.tile([65, HF], f32, tag="cx", name=f"cx_{c}_{hh}_{z}")
                    for z in range(2)
                ]
                pts = [None] * SK

                kp_sb = kpA_sb if hh == 0 else kpB_sb

                def qk(i):
                    ksl = slice(i * 128, (i + 1) * 128)
                    ss = sp.tile([128, CH], f32, tag="ss", name=f"ss_{c}_{hh}_{i}")
                    for z in range(2):
                        nc.tensor.matmul(
                            ss[:, z * HF : (z + 1) * HF],
                            kp_sb[:, ksl],
                            qT_sb[:, cs0 + z * HF : cs0 + (z + 1) * HF],
                            start=True,
                            stop=True,
                        )
                    pt = ptp.tile([128, CH], MM, tag="pt", name=f"pt_{c}_{hh}_{i}")
                    nc.scalar.activation(pt, ss, Exp, scale=0.125)
                    pts[i] = pt

                w1_at = min(6, SK - 2)
                w2_at = min(10, SK - 1)
                qk(0)
                for i in range(SK):
                    if i + 1 < SK:
                        qk(i + 1)
                    if i == w1_at and work1 is not None:
                        work1()
                    if i == w2_at and work2 is not None:
                        work2()
                    for z in range(2):
                        nc.tensor.matmul(
                            cx[z],
                            v1_sb[:, hh, i, :],
                            pts[i][:, z * HF : (z + 1) * HF],
                            start=(i == 0),
                            stop=(i == SK - 1),
                        )

                # stage the unnormalized context (and its denominator row 64)
                # out of PSUM right away — frees cx banks for the next head
                stage = mss.tile([65, CH], f32, tag="stage", name=f"stage_{c}_{hh}")
                for z in range(2):
                    nc.vector.tensor_copy(stage[:, z * HF : (z + 1) * HF], cx[z])
                # reciprocal on the (otherwise idle-at-this-point) ACT engine:
                # rec = exp(-ln(den)); Ln and Exp share one table set
                lnb = mss.tile([1, CH], f32, tag="lnb", name=f"lnb_{c}_{hh}")
                nc.scalar.activation(lnb, stage[64:65, :], Ln)
                rec = mss.tile([1, CH], f32, tag="rec", name=f"rec_{c}_{hh}")
                nc.scalar.activation(rec, lnb, Exp, scale=-1.0)
                rb = mss.tile([64, CH], f32, tag="rb", name=f"rb_{c}_{hh}")

                def epi():
                    for z in range(2):
                        rp_ps = msp.tile(
                            [64, HF], f32, tag="mm", name=f"rp_{c}_{hh}_{z}"
                        )
                        # K=1 broadcast matmul in plain fp32 (rec spans values
                        # near the fp16 subnormal range — keep it f32)
                        nc.tensor.matmul(
                            rp_ps,
                            ones_sb,
                            rec[:, z * HF : (z + 1) * HF],
                            start=True,
                            stop=True,
                        )
                        nc.vector.tensor_copy(rb[:, z * HF : (z + 1) * HF], rp_ps)
                    # normalize; cast into the MM-typed ctx tensor
                    nc.vector.tensor_mul(
                        ctx_sb[hsl, cs0 : cs0 + CH], stage[0:64, :], rb
                    )

                return epi

            # stream schedule: each head's epilogue runs inside the next
            # stream (slot 1, at i==6); the chunk's o_proj is split across the
            # next two streams (slot 1 / slot 2 at i==14) as dense PE bursts
            # that double as HAM warmers. The first streams get dummy bursts.
            pend1 = [lambda: emit_warmer("w0")]
            pend2 = [lambda: emit_warmer("w1"), lambda: emit_warmer("w2")]

            def take(lst):
                return lst.pop(0) if lst else None

            for c in range(NCH):
                for hh in range(2):
                    epi = emit_head(c, hh, take(pend1), take(pend2))
                    if hh == 0:
                        pend1.append(epi)
                    else:
                        cc = c

                        nsq = CH // 128

                        def epi_and_first_oproj(e=epi, cc=cc):
                            e()
                            emit_oproj(cc, range(0, nsq // 2))

                        pend1.append(epi_and_first_oproj)
                        # lands two streams later (the following B stream)
                        pend2.append(None)
                        pend2.append(lambda cc=cc: emit_oproj(cc, range(nsq // 2, nsq)))
            # drain remaining deferred work after the last stream
            for w in pend1 + pend2:
                if w is not None:
                    w()
    return nc


def _legalize_sync_waits(nc, max_waits=1):
    """Cap sync waits per instruction for this container's walrus build.

    The bundled walrus encodes a limited number of sync-wait commands per
    instruction ("Too many sync wait commands" codegen error), while Tile
    attaches one wait per logical processor where needed. An attached wait
    is equivalent to a standalone preceding wait on the same engine (that
    is exactly what raw-bass `wait_ge` emits: a pure-wait
    InstEventSemaphore), so hoist the excess waits onto EventSemaphore
    instructions inserted right before the offender.
    """
    from concourse import mybir

    n_fixed = 0
    for fn in nc.m.functions:
        for b in fn.blocks:
            insts = b.instructions
            idx = 0
            while idx < len(insts):
                inst = insts[idx]
                si = inst.sync_info
                waits = list(si.on_wait) if si and si.on_wait else []
                if len(waits) > max_waits:
                    updates = list(si.on_update) if si and si.on_update else []
                    pre, keep = waits[: -max_waits], waits[-max_waits:]
                    clones = []
                    for j, w in enumerate(pre):
                        clones.append(
                            mybir.InstEventSemaphore(
                                name=f"{inst.name}_sw{j}",
                                engine=inst.engine,
                                ins=[],
                                outs=[],
                                sync_info=mybir.SyncInfo(on_wait=[w], on_update=[]),
                            )
                        )
                    inst.sync_info = mybir.SyncInfo(on_wait=keep, on_update=updates)
                    for j, clone in enumerate(clones):
                        insts.insert(idx + j, clone)
                        try:
                            nc.inst_map[clone.name] = clone
                        except Exception:
                            pass
                    idx += len(clones)
                    n_fixed += 1
                idx += 1
    return n_fixed


def get_nc(S=SEQ, mm_dt="float16"):
    key = (S, mm_dt)
    if key not in _NC_CACHE:
        nc = _build_nc(S, mm_dt)
        _legalize_sync_waits(nc)
        _NC_CACHE[key] = nc
    return _NC_CACHE[key]


def _mm_np_dtype(mm_dt):
    if mm_dt == "bfloat16":
        import ml_dtypes

        return np.dtype(ml_dtypes.bfloat16)
    if mm_dt == "float16":
        return np.dtype(np.float16)
    return np.dtype(np.float32)  # float32r carries fp32 bits


def make_in_maps(h, cos, sin, wq, bq, wk, bk, wv, bv, wo, mm_dt="float32r"):
    """Host-side shard prep. h [B,S,HID] -> per-core input dict."""
    mdt = _mm_np_dtype(mm_dt)
    h = np.asarray(h, dtype=np.float32)
    S = h.shape[1]
    hT = np.ascontiguousarray(h[0].T).astype(mdt)  # [HID, S]
    cos = np.asarray(cos, dtype=np.float32)
    sin = np.asarray(sin, dtype=np.float32)
    cosT = np.ascontiguousarray(cos.T)  # [64, S]
    sinT = sin.T
    # rotate_half: q'[0:32] = q[:32]*cos - q[32:64]*sin[0:32]
    #              q'[32:64] = q[32:64]*cos + q[0:32]*sin[32:64]
    # The kernel computes tmp[da] = q[sa] * sinTs[sa] with (da,sa) row-halves
    # swapped, so the table carries the destination row's signed sin at the
    # source row: sinTs[0:32] = +sin[32:64].T, sinTs[32:64] = -sin[0:32].T.
    sinTs = np.ascontiguousarray(
        np.concatenate([sinT[HEAD_DIM // 2 :], -sinT[: HEAD_DIM // 2]], axis=0)
    )
    wq = np.asarray(wq, dtype=np.float32)
    wk = np.asarray(wk, dtype=np.float32)
    wv = np.asarray(wv, dtype=np.float32)
    wo = np.asarray(wo, dtype=np.float32)
    bq = np.asarray(bq, dtype=np.float32)
    bk = np.asarray(bk, dtype=np.float32)
    bv = np.asarray(bv, dtype=np.float32)
    in_maps = []
    for c in range(NCORES):
        fs = slice(c * FPC, (c + 1) * FPC)
        in_maps.append(
            {
                "hT": hT,
                "wqT": np.ascontiguousarray(wq[fs, :].T).astype(mdt),
                "wkT": np.ascontiguousarray(wk[fs, :].T).astype(mdt),
                "wvT": np.ascontiguousarray(wv[fs, :].T).astype(mdt),
                "bqkv": np.ascontiguousarray(
                    np.stack([bq[fs], bk[fs], bv[fs]], axis=1)
                ),
                "woT": np.ascontiguousarray(wo[:, fs].T).astype(mdt),
                "cosT": cosT,
                "sinTs": sinTs,
            }
        )
    return in_maps


MM_DT = "float16"


def kernel(h, mask, cos, sin, wq, bq, wk, bk, wv, bv, wo, bo, **_unused):
    # mask is all-ones per the problem spec; post-softmax where(mask==0) is a no-op.
    from concourse.bass_utils import run_bass_kernel_spmd

    h = np.asarray(h, dtype=np.float32)
    S = h.shape[1]
    nc = get_nc(S, MM_DT)
    in_maps = make_in_maps(h, cos, sin, wq, bq, wk, bk, wv, bv, wo, MM_DT)
    res = run_bass_kernel_spmd(nc, in_maps, core_ids=list(range(NCORES)))
    acc = np.zeros((S, HIDDEN), dtype=np.float32)
    for r in res.results:
        acc += r["out"]
    acc += np.asarray(bo, dtype=np.float32)[None, :]
    return acc[None].astype(np.float32)



# revision 9
# speedup vs baseline: 1.3185x; 1.3185x over previous
"""Trainium2 Bass kernel for MHA with RoPE (dense transformer block).

Problem shapes: h [1, 4096, 1024], 16 heads x 64 dim, full (non-causal)
softmax attention, post-softmax all-ones mask (identity), torch-Linear
projections with bias.

Sharding: head-parallel across 8 cores (2 heads/core). v2 design:

Prologue (projections, ~50us):
  - q/k/v projections from fp16 hT with weights stationary.
  - RoPE via PE rotation matmul: qs = R*(q+b) where R is the
    rotate-half permutation as an fp16 [128,128] matrix; then on DVE
    q' = (q+b) o cos + qs o sin (bias folded into ACT staging copy).
  - v transposed to key-major via fp16 PE transpose; bias bv is folded
    into bo on the host (softmax rows sum to 1 exactly).

Attention (PE/ACT/DVE balanced, ~185us):
  - per 512-query chunk x 32 key tiles: row-tiled QK pair (K=64 per
    head at tile_position (0,0)/(64,0), both heads concurrent, no
    zero-padding), one [128,1024] psum score tile per iteration.
  - exp alternates between ACT (table exp, scale=1/8) and DVE
    (Schraudolph: fp16 bits = int16(score*A + B), one tensor_scalar,
    bitcast free) halving the softmax wall time.
  - PV with a ones-row appended to v (denominator for free, M=65).
  - normalize with rec = exp(-ln(den) + ln 4096) on ACT (scaled to
    stay in fp16 normal range; host divides by 4096), broadcast via
    K=1 ones matmul, applied by DVE.
  - o_proj partials DMA'd straight from PSUM to DRAM fp32.

Host sums the 8 partial outputs, divides by 4096, adds bo + wo@bv.
"""

import math

import numpy as np

HIDDEN = 1024
HEADS = 16
HEAD_DIM = 64
SEQ = 4096
NCORES = 8
FPC = 128  # features per core = 2 heads x 64

# Schraudolph fp16-exp constants: fp16bits(e^(s/8)) ~= int16(s*EXP_A + EXP_B)
EXP_A = 1024.0 * 0.125 / math.log(2.0)
EXP_B = 15360.0 - 44.0

_NC_CACHE = {}


def _build_nc(S=SEQ):
    import concourse.bass as bass
    import concourse.tile as tile
    from concourse import mybir
    from contextlib import ExitStack

    f32 = mybir.dt.float32
    f16 = mybir.dt.float16
    i16 = mybir.dt.int16
    Exp = mybir.ActivationFunctionType.Exp
    Ln = mybir.ActivationFunctionType.Ln
    Ident = mybir.ActivationFunctionType.Identity
    Mult = mybir.AluOpType.mult
    Add = mybir.AluOpType.add

    D = HEAD_DIM
    HID = HIDDEN
    KT = HID // 128  # hidden contraction tiles
    PC = 512         # projection seq chunk
    NPC = S // PC
    CH = 512         # attention query chunk
    NCH = S // CH
    SK = S // 128    # key tiles
    PVLAG = 2        # software-pipeline lag of PV behind QK/exp

    nc = bass.Bass(trn_type="TRN2")

    hT = nc.dram_tensor("hT", [HID, S], f16, kind="ExternalInput")
    wqT = nc.dram_tensor("wqT", [HID, FPC], f16, kind="ExternalInput")
    wkT = nc.dram_tensor("wkT", [HID, FPC], f16, kind="ExternalInput")
    wvT = nc.dram_tensor("wvT", [HID, FPC], f16, kind="ExternalInput")
    bqk = nc.dram_tensor("bqk", [FPC, 2], f32, kind="ExternalInput")
    woT = nc.dram_tensor("woT", [FPC, HID], f16, kind="ExternalInput")
    cosT = nc.dram_tensor("cosT", [D, S], f16, kind="ExternalInput")
    sinT = nc.dram_tensor("sinT", [D, S], f16, kind="ExternalInput")
    out = nc.dram_tensor("out", [S, HID], f16, kind="ExternalOutput")

    hT3 = hT[:, :].rearrange("(ko p) s -> p ko s", p=128)

    # rotate-half as a matmul: qs = R @ q with R = blockdiag([[0,-I],[I,0]]).
    # matmul computes lhsT.T @ rhs so we feed R^T = blockdiag([[0,I],[-I,0]]).
    rotT_np = np.zeros((FPC, FPC), dtype=np.float16)
    for hh in range(2):
        o = hh * 64
        for j in range(32):
            rotT_np[o + 32 + j, o + j] = -1.0
            rotT_np[o + j, o + 32 + j] = 1.0
    rotT_dram = nc.inline_tensor(rotT_np, name="rotT")
    ident_np = np.eye(128, dtype=np.float16)
    ident_dram = nc.inline_tensor(ident_np, name="ident16")

    with tile.TileContext(nc) as tc, ExitStack() as top:
        sing = top.enter_context(tc.tile_pool(name="sing", bufs=1))

        wq_sb = sing.tile([128, KT, FPC], f16)
        wk_sb = sing.tile([128, KT, FPC], f16)
        wv_sb = sing.tile([128, KT, FPC], f16)
        nc.sync.dma_start(wq_sb, wqT[:, :].rearrange("(ko p) f -> p ko f", p=128))
        nc.sync.dma_start(wk_sb, wkT[:, :].rearrange("(ko p) f -> p ko f", p=128))
        nc.sync.dma_start(wv_sb, wvT[:, :].rearrange("(ko p) f -> p ko f", p=128))
        wo_sb = sing.tile([FPC, HID], f16)
        nc.sync.dma_start(wo_sb, woT[:, :])
        b_sb = sing.tile([FPC, 2], f32)
        nc.sync.dma_start(b_sb, bqk[:, :])
        cos_sb = sing.tile([128, S], f16)
        sin_sb = sing.tile([128, S], f16)
        nc.sync.dma_start(cos_sb[0:64, :], cosT[:, :])
        nc.sync.dma_start(cos_sb[64:128, :], cosT[:, :])
        nc.sync.dma_start(sin_sb[0:64, :], sinT[:, :])
        nc.sync.dma_start(sin_sb[64:128, :], sinT[:, :])
        rot_sb = sing.tile([FPC, FPC], f16)
        nc.sync.dma_start(rot_sb, rotT_dram[:, :])
        ident_sb = sing.tile([128, 128], f16)
        nc.sync.dma_start(ident_sb, ident_dram[:, :])
        ones_sb = sing.tile([1, 64], f16)
        nc.vector.memset(ones_sb, 1.0)
        ln4096_sb = sing.tile([1, 1], f32)
        nc.vector.memset(ln4096_sb, math.log(4096.0))

        qT_sb = sing.tile([128, S], f16)
        kp_sb = sing.tile([128, S], f16)
        # v1[:, hh, i, 0:64] = v tile (keys-major); [.., 64] = ones row so the
        # PV matmul also produces the softmax denominator.
        v1_sb = sing.tile([128, 2, SK, 65], f16)
        nc.vector.memset(v1_sb, 1.0)
        ctx_sb = sing.tile([128, S], f16)

        # ---- projections + RoPE + v transpose ----
        with ExitStack() as ph1:
            hp = ph1.enter_context(tc.tile_pool(name="hp", bufs=2))
            rp = ph1.enter_context(tc.tile_pool(name="rope", bufs=8))
            pps = ph1.enter_context(tc.tile_pool(name="pps", bufs=4, space="PSUM"))
            qsp = ph1.enter_context(tc.tile_pool(name="qsp", bufs=2, space="PSUM"))
            tps = ph1.enter_context(tc.tile_pool(name="tps", bufs=2, space="PSUM"))
            for ch in range(NPC):
                ssl = slice(ch * PC, (ch + 1) * PC)
                h_sb = hp.tile([128, KT, PC], f16)
                nc.sync.dma_start(h_sb, hT3[:, :, ssl])
                for wi, (w_sb, dst) in enumerate(
                    [(wq_sb, qT_sb), (wk_sb, kp_sb), (wv_sb, None)]
                ):
                    ps = pps.tile([128, PC], f32, tag="ps", name=f"ps_{ch}_{wi}")
                    for k in range(KT):
                        nc.tensor.matmul(
                            ps,
                            w_sb[:, k, :],
                            h_sb[:, k, :],
                            start=(k == 0),
                            stop=(k == KT - 1),
                        )
                    if dst is not None:
                        # stage with bias on ACT (psum -> fp16 sbuf)
                        stg = rp.tile([128, PC], f16, tag="stg", name=f"stg_{ch}_{wi}")
                        nc.scalar.activation(
                            stg, ps, Ident, bias=b_sb[:, wi : wi + 1]
                        )
                        qs = qsp.tile([128, PC], f32, tag="qs", name=f"qs_{ch}_{wi}")
                        nc.tensor.matmul(qs, rot_sb, stg, start=True, stop=True)
                        t1 = rp.tile([128, PC], f16, tag="t1", name=f"t1_{ch}_{wi}")
                        nc.vector.tensor_mul(t1, stg, cos_sb[:, ssl])
                        t2 = rp.tile([128, PC], f16, tag="t2", name=f"t2_{ch}_{wi}")
                        nc.vector.tensor_mul(t2, qs, sin_sb[:, ssl])
                        nc.vector.tensor_add(dst[:, ssl], t1, t2)
                    else:
                        stgv = rp.tile([128, PC], f16, tag="stgv", name=f"stgv_{ch}")
                        nc.scalar.activation(stgv, ps, Ident)
                        for st in range(PC // 128):
                            kti = ch * (PC // 128) + st
                            tp = tps.tile(
                                [128, 128], f16, tag="tp", name=f"tp_{ch}_{st}"
                            )
                            nc.tensor.transpose(
                                tp, stgv[:, st * 128 : (st + 1) * 128], ident_sb
                            )
                            nc.vector.tensor_copy(v1_sb[:, :, kti, 0:64], tp)

        # ---- attention + o_proj ----
        with ExitStack() as ph2:
            ssp = ph2.enter_context(tc.tile_pool(name="ssp", bufs=2, space="PSUM"))
            cxp = ph2.enter_context(tc.tile_pool(name="cxp", bufs=1, space="PSUM"))
            opp = ph2.enter_context(tc.tile_pool(name="opp", bufs=2, space="PSUM"))
            ptp = ph2.enter_context(tc.tile_pool(name="ptp", bufs=6))
            mss = ph2.enter_context(tc.tile_pool(name="mss", bufs=8))

            def emit_oproj(c, sqs):
                cs0 = c * CH
                for sq in sqs:
                    r0 = cs0 + sq * 128
                    for nz in range(HID // 512):
                        ops = opp.tile(
                            [128, 512], f32, tag="op", name=f"op_{c}_{sq}_{nz}"
                        )
                        nc.tensor.matmul(
                            ops,
                            ctx_sb[:, r0 : r0 + 128],
                            wo_sb[:, nz * 512 : (nz + 1) * 512],
                            start=True,
                            stop=True,
                        )
                        ob = mss.tile(
                            [128, 512], f16, tag="ob", name=f"ob_{c}_{sq}_{nz}"
                        )
                        # psum->sbuf fp16 staging, alternating engine
                        if nz % 2 == 0:
                            nc.vector.tensor_copy(ob, ops)
                        else:
                            nc.scalar.activation(ob, ops, Ident)
                        nc.sync.dma_start(
                            out[r0 : r0 + 128, nz * 512 : (nz + 1) * 512], ob
                        )

            cx = [None, None]
            pts = [None] * SK

            def emit_qk_exp(c, i):
                cs0 = c * CH
                ksl = slice(i * 128, (i + 1) * 128)
                csl = slice(cs0, cs0 + CH)
                ss = ssp.tile([128, 2 * CH], f32, tag="ss", name=f"ss_{c}_{i}")
                for hh in range(2):
                    psl = slice(hh * 64, hh * 64 + 64)
                    nc.tensor.matmul(
                        ss[:, hh * CH : (hh + 1) * CH],
                        kp_sb[psl, ksl],
                        qT_sb[psl, csl],
                        start=True,
                        stop=True,
                    )
                pt = ptp.tile([128, 2 * CH], f16, tag="pt", name=f"pt_{c}_{i}")
                if (i + c) % 2 == 0:
                    nc.scalar.activation(pt, ss, Exp, scale=0.125)
                else:
                    nc.vector.tensor_scalar(
                        pt[:, :].bitcast(i16), ss, EXP_A, EXP_B, Mult, Add
                    )
                pts[i] = pt

            def emit_pv(c, i):
                for hh in range(2):
                    nc.tensor.matmul(
                        cx[hh],
                        v1_sb[:, hh, i, :],
                        pts[i][:, hh * CH : (hh + 1) * CH],
                        start=(i == 0),
                        stop=(i == SK - 1),
                    )

            def emit_epilogue(c):
                cs0 = c * CH
                for hh in range(2):
                    hsl = slice(hh * 64, hh * 64 + 64)
                    stage = mss.tile(
                        [65, CH], f16, tag="stage", name=f"stage_{c}_{hh}"
                    )
                    nc.vector.tensor_copy(stage, cx[hh])
                    lnb = mss.tile([1, CH], f32, tag="lnb", name=f"lnb_{c}_{hh}")
                    nc.scalar.activation(lnb, stage[64:65, :], Ln)
                    rec = mss.tile([1, CH], f16, tag="rec", name=f"rec_{c}_{hh}")
                    # rec = 4096/den keeps fp16 in normal range; host undoes it
                    nc.scalar.activation(
                        rec, lnb, Exp, scale=-1.0, bias=ln4096_sb[:, :]
                    )
                    rb = opp.tile([128, CH], f32, tag="op", name=f"rb_{c}_{hh}")
                    nc.tensor.matmul(rb[0:64, :], ones_sb, rec, start=True, stop=True)
                    nc.vector.tensor_mul(
                        ctx_sb[hsl, cs0 : cs0 + CH], stage[0:64, :], rb[0:64, :]
                    )

            for c in range(NCH):
                cx[0] = cxp.tile([65, CH], f32, tag="cxA", name=f"cxA_{c}")
                cx[1] = cxp.tile([65, CH], f32, tag="cxB", name=f"cxB_{c}")
                for i in range(SK):
                    emit_qk_exp(c, i)
                    if i >= PVLAG:
                        emit_pv(c, i - PVLAG)
                    if c > 0 and i == 10:
                        emit_oproj(c - 1, range(0, 2))
                    if c > 0 and i == 20:
                        emit_oproj(c - 1, range(2, 4))
                for i in range(SK - PVLAG, SK):
                    emit_pv(c, i)
                emit_epilogue(c)
            emit_oproj(NCH - 1, range(0, 4))
    return nc


def _legalize_sync_waits(nc, max_waits=1):
    """Cap sync waits per instruction for this container's walrus build.

    The bundled walrus encodes a limited number of sync-wait commands per
    instruction ("Too many sync wait commands" codegen error), while Tile
    attaches one wait per logical processor where needed. An attached wait
    is equivalent to a standalone preceding wait on the same engine (that
    is exactly what raw-bass `wait_ge` emits: a pure-wait
    InstEventSemaphore), so hoist the excess waits onto EventSemaphore
    instructions inserted right before the offender.
    """
    from concourse import mybir

    n_fixed = 0
    for fn in nc.m.functions:
        for b in fn.blocks:
            insts = b.instructions
            idx = 0
            while idx < len(insts):
                inst = insts[idx]
                si = inst.sync_info
                waits = list(si.on_wait) if si and si.on_wait else []
                if len(waits) > max_waits:
                    updates = list(si.on_update) if si and si.on_update else []
                    pre, keep = waits[: -max_waits], waits[-max_waits:]
                    clones = []
                    for j, w in enumerate(pre):
                        clones.append(
                            mybir.InstEventSemaphore(
                                name=f"{inst.name}_sw{j}",
                                engine=inst.engine,
                                ins=[],
                                outs=[],
                                sync_info=mybir.SyncInfo(on_wait=[w], on_update=[]),
                            )
                        )
                    inst.sync_info = mybir.SyncInfo(on_wait=keep, on_update=updates)
                    for j, clone in enumerate(clones):
                        insts.insert(idx + j, clone)
                        try:
                            nc.inst_map[clone.name] = clone
                        except Exception:
                            pass
                    idx += len(clones)
                    n_fixed += 1
                idx += 1
    return n_fixed


MM_DT = "float16"


def get_nc(S=SEQ, mm_dt=MM_DT):
    key = S
    if key not in _NC_CACHE:
        nc = _build_nc(S)
        _legalize_sync_waits(nc)
        _NC_CACHE[key] = nc
    return _NC_CACHE[key]


def make_in_maps(h, cos, sin, wq, bq, wk, bk, wv, bv, wo):
    """Host-side shard prep. h [B,S,HID] -> per-core input dict."""
    f16 = np.float16
    h = np.asarray(h, dtype=np.float32)
    S = h.shape[1]
    hT = np.ascontiguousarray(h[0].T).astype(f16)  # [HID, S]
    cosT = np.ascontiguousarray(np.asarray(cos, np.float32).T).astype(f16)
    sinT = np.ascontiguousarray(np.asarray(sin, np.float32).T).astype(f16)
    wq = np.asarray(wq, dtype=np.float32)
    wk = np.asarray(wk, dtype=np.float32)
    wv = np.asarray(wv, dtype=np.float32)
    wo = np.asarray(wo, dtype=np.float32)
    bq = np.asarray(bq, dtype=np.float32)
    bk = np.asarray(bk, dtype=np.float32)
    in_maps = []
    for c in range(NCORES):
        fs = slice(c * FPC, (c + 1) * FPC)
        in_maps.append(
            {
                "hT": hT,
                "wqT": np.ascontiguousarray(wq[fs, :].T).astype(f16),
                "wkT": np.ascontiguousarray(wk[fs, :].T).astype(f16),
                "wvT": np.ascontiguousarray(wv[fs, :].T).astype(f16),
                "bqk": np.ascontiguousarray(
                    np.stack([bq[fs], bk[fs]], axis=1).astype(np.float32)
                ),
                "woT": np.ascontiguousarray(wo[:, fs].T).astype(f16),
                "cosT": cosT,
                "sinT": sinT,
            }
        )
    return in_maps


def kernel(h, mask, cos, sin, wq, bq, wk, bk, wv, bv, wo, bo, **_unused):
    # mask is all-ones per the problem spec; post-softmax where(mask==0) is a no-op.
    from concourse.bass_utils import run_bass_kernel_spmd

    h = np.asarray(h, dtype=np.float32)
    S = h.shape[1]
    nc = get_nc(S)
    in_maps = make_in_maps(h, cos, sin, wq, bq, wk, bk, wv, bv, wo)
    res = run_bass_kernel_spmd(nc, in_maps, core_ids=list(range(NCORES)))
    acc = np.zeros((S, HIDDEN), dtype=np.float64)
    for r in res.results:
        acc += r["out"].astype(np.float64)
    acc /= 4096.0
    bo_eff = np.asarray(bo, np.float64) + np.asarray(wo, np.float64) @ np.asarray(
        bv, np.float64
    )
    acc += bo_eff[None, :]
    return acc[None].astype(np.float32)


# revision 14
# speedup vs baseline: 1.3228x; 1.0033x over previous
"""Trainium2 Bass kernel for MHA with RoPE (dense transformer block).

Problem shapes: h [1, 4096, 1024], 16 heads x 64 dim, full (non-causal)
softmax attention, post-softmax all-ones mask (identity), torch-Linear
projections with bias.

Sharding: head-parallel across 8 cores (2 heads/core). v2 design:

Prologue (projections, ~50us):
  - q/k/v projections from fp16 hT with weights stationary.
  - RoPE via PE rotation matmul: qs = R*(q+b) where R is the
    rotate-half permutation as an fp16 [128,128] matrix; then on DVE
    q' = (q+b) o cos + qs o sin (bias folded into ACT staging copy).
  - v transposed to key-major via fp16 PE transpose; bias bv is folded
    into bo on the host (softmax rows sum to 1 exactly).

Attention (PE/ACT/DVE balanced, ~185us):
  - per 512-query chunk x 32 key tiles: row-tiled QK pair (K=64 per
    head at tile_position (0,0)/(64,0), both heads concurrent, no
    zero-padding), one [128,1024] psum score tile per iteration.
  - exp alternates between ACT (table exp, scale=1/8) and DVE
    (Schraudolph: fp16 bits = int16(score*A + B), one tensor_scalar,
    bitcast free) halving the softmax wall time.
  - PV with a ones-row appended to v (denominator for free, M=65).
  - normalize with rec = exp(-ln(den) + ln 4096) on ACT (scaled to
    stay in fp16 normal range; host divides by 4096), broadcast via
    K=1 ones matmul, applied by DVE.
  - o_proj partials DMA'd straight from PSUM to DRAM fp32.

Host sums the 8 partial outputs, divides by 4096, adds bo + wo@bv.
"""

import math

import numpy as np

HIDDEN = 1024
HEADS = 16
HEAD_DIM = 64
SEQ = 4096
NCORES = 8
FPC = 128  # features per core = 2 heads x 64

# Schraudolph fp16-exp constants: fp16bits(e^(s/8)) ~= int16(s*EXP_A + EXP_B)
EXP_A = 1024.0 * 0.125 / math.log(2.0)
EXP_B = 15360.0 - 44.0

_NC_CACHE = {}


def _build_nc(S=SEQ):
    import concourse.bass as bass
    import concourse.tile as tile
    from concourse import mybir
    from contextlib import ExitStack

    f32 = mybir.dt.float32
    f16 = mybir.dt.float16
    i16 = mybir.dt.int16
    Exp = mybir.ActivationFunctionType.Exp
    Ln = mybir.ActivationFunctionType.Ln
    Ident = mybir.ActivationFunctionType.Identity
    Mult = mybir.AluOpType.mult
    Add = mybir.AluOpType.add

    D = HEAD_DIM
    HID = HIDDEN
    KT = HID // 128  # hidden contraction tiles
    PC = 512         # projection seq chunk
    NPC = S // PC
    CH = 512         # attention query chunk
    NCH = S // CH
    SK = S // 128    # key tiles
    PVLAG = 3        # software-pipeline lag of PV behind QK/exp

    nc = bass.Bass(trn_type="TRN2")

    hT = nc.dram_tensor("hT", [HID, S], f16, kind="ExternalInput")
    wqT = nc.dram_tensor("wqT", [HID, FPC], f16, kind="ExternalInput")
    wkT = nc.dram_tensor("wkT", [HID, FPC], f16, kind="ExternalInput")
    wvT = nc.dram_tensor("wvT", [HID, FPC], f16, kind="ExternalInput")
    bqk = nc.dram_tensor("bqk", [FPC, 2], f32, kind="ExternalInput")
    woT = nc.dram_tensor("woT", [FPC, HID], f16, kind="ExternalInput")
    cosT = nc.dram_tensor("cosT", [D, S], f16, kind="ExternalInput")
    sinT = nc.dram_tensor("sinT", [D, S], f16, kind="ExternalInput")
    out = nc.dram_tensor("out", [S, HID], f16, kind="ExternalOutput")

    hT3 = hT[:, :].rearrange("(ko p) s -> p ko s", p=128)

    # rotate-half as a matmul: qs = R @ q with R = blockdiag([[0,-I],[I,0]]).
    # matmul computes lhsT.T @ rhs so we feed R^T = blockdiag([[0,I],[-I,0]]).
    rotT_np = np.zeros((FPC, FPC), dtype=np.float16)
    for hh in range(2):
        o = hh * 64
        for j in range(32):
            rotT_np[o + 32 + j, o + j] = -1.0
            rotT_np[o + j, o + 32 + j] = 1.0
    rotT_dram = nc.inline_tensor(rotT_np, name="rotT")
    ident_np = np.eye(128, dtype=np.float16)
    ident_dram = nc.inline_tensor(ident_np, name="ident16")

    with tile.TileContext(nc) as tc, ExitStack() as top:
        sing = top.enter_context(tc.tile_pool(name="sing", bufs=1))

        wq_sb = sing.tile([128, KT, FPC], f16)
        wk_sb = sing.tile([128, KT, FPC], f16)
        wv_sb = sing.tile([128, KT, FPC], f16)
        nc.sync.dma_start(wq_sb, wqT[:, :].rearrange("(ko p) f -> p ko f", p=128))
        nc.sync.dma_start(wk_sb, wkT[:, :].rearrange("(ko p) f -> p ko f", p=128))
        nc.sync.dma_start(wv_sb, wvT[:, :].rearrange("(ko p) f -> p ko f", p=128))
        wo_sb = sing.tile([FPC, HID], f16)
        nc.sync.dma_start(wo_sb, woT[:, :])
        b_sb = sing.tile([FPC, 2], f32)
        nc.sync.dma_start(b_sb, bqk[:, :])
        cos_sb = sing.tile([128, S], f16)
        sin_sb = sing.tile([128, S], f16)
        nc.sync.dma_start(cos_sb[0:64, :], cosT[:, :])
        nc.sync.dma_start(cos_sb[64:128, :], cosT[:, :])
        nc.sync.dma_start(sin_sb[0:64, :], sinT[:, :])
        nc.sync.dma_start(sin_sb[64:128, :], sinT[:, :])
        rot_sb = sing.tile([FPC, FPC], f16)
        nc.sync.dma_start(rot_sb, rotT_dram[:, :])
        ident_sb = sing.tile([128, 128], f16)
        nc.sync.dma_start(ident_sb, ident_dram[:, :])
        ones_sb = sing.tile([1, 64], f16)
        nc.vector.memset(ones_sb, 1.0)
        ln4096_sb = sing.tile([1, 1], f32)
        nc.vector.memset(ln4096_sb, math.log(4096.0))

        # warm the natural_log_exp table set during the prologue so the
        # first attention exp doesn't eat the ~2.7us ACT_TABLE_LOAD
        warm_sb = sing.tile([1, 1], f32)
        nc.scalar.activation(warm_sb, ln4096_sb, Ln)
        nc.scalar.activation(warm_sb, warm_sb, Exp)

        qT_sb = sing.tile([128, S], f16)
        kp_sb = sing.tile([128, S], f16)
        # v1[:, hh, i, 0:64] = v tile (keys-major); [.., 64] = ones row so the
        # PV matmul also produces the softmax denominator.
        v1_sb = sing.tile([128, 2, SK, 65], f16)
        nc.vector.memset(v1_sb, 1.0)
        ctx_sb = sing.tile([128, S], f16)

        # ---- projections + RoPE + v transpose ----
        with ExitStack() as ph1:
            hp = ph1.enter_context(tc.tile_pool(name="hp", bufs=2))
            rp = ph1.enter_context(tc.tile_pool(name="rope", bufs=8))
            pps = ph1.enter_context(tc.tile_pool(name="pps", bufs=4, space="PSUM"))
            qsp = ph1.enter_context(tc.tile_pool(name="qsp", bufs=2, space="PSUM"))
            tps = ph1.enter_context(tc.tile_pool(name="tps", bufs=2, space="PSUM"))
            # Software-pipelined: each projection group's PE epilogue (the
            # rotation matmul / v transposes, which wait on an ACT staging
            # copy) is emitted under the NEXT group's matmul stream so the
            # PE never stalls on ACT latency.
            pend = []

            def rope_tail(ch, wi, stg, dst):
                ssl = slice(ch * PC, (ch + 1) * PC)
                qs = qsp.tile([128, PC], f32, tag="qs", name=f"qs_{ch}_{wi}")
                nc.tensor.matmul(qs, rot_sb, stg, start=True, stop=True)
                t1 = rp.tile([128, PC], f16, tag="t1", name=f"t1_{ch}_{wi}")
                nc.vector.tensor_mul(t1, stg, cos_sb[:, ssl])
                t2 = rp.tile([128, PC], f16, tag="t2", name=f"t2_{ch}_{wi}")
                nc.vector.tensor_mul(t2, qs, sin_sb[:, ssl])
                nc.vector.tensor_add(dst[:, ssl], t1, t2)

            def v_tail(ch, stgv):
                for st in range(PC // 128):
                    kti = ch * (PC // 128) + st
                    tp = tps.tile([128, 128], f16, tag="tp", name=f"tp_{ch}_{st}")
                    nc.tensor.transpose(
                        tp, stgv[:, st * 128 : (st + 1) * 128], ident_sb
                    )
                    nc.vector.tensor_copy(v1_sb[:, :, kti, 0:64], tp)

            for ch in range(NPC):
                ssl = slice(ch * PC, (ch + 1) * PC)
                h_sb = hp.tile([128, KT, PC], f16)
                nc.sync.dma_start(h_sb, hT3[:, :, ssl])
                for wi, (w_sb, dst) in enumerate(
                    [(wq_sb, qT_sb), (wk_sb, kp_sb), (wv_sb, None)]
                ):
                    ps = pps.tile([128, PC], f32, tag="ps", name=f"ps_{ch}_{wi}")
                    for k in range(KT):
                        nc.tensor.matmul(
                            ps,
                            w_sb[:, k, :],
                            h_sb[:, k, :],
                            start=(k == 0),
                            stop=(k == KT - 1),
                        )
                    if pend:
                        pend.pop(0)()
                    if dst is not None:
                        # stage with bias on ACT (psum -> fp16 sbuf)
                        stg = rp.tile([128, PC], f16, tag="stg", name=f"stg_{ch}_{wi}")
                        nc.scalar.activation(
                            stg, ps, Ident, bias=b_sb[:, wi : wi + 1]
                        )
                        pend.append(
                            lambda ch=ch, wi=wi, stg=stg, dst=dst: rope_tail(
                                ch, wi, stg, dst
                            )
                        )
                    else:
                        stgv = rp.tile([128, PC], f16, tag="stgv", name=f"stgv_{ch}")
                        nc.scalar.activation(stgv, ps, Ident)
                        pend.append(lambda ch=ch, stgv=stgv: v_tail(ch, stgv))
            for w in pend:
                w()

        # ---- attention + o_proj ----
        with ExitStack() as ph2:
            # ss ring (3 slots) shares its tag with o_proj/rec-broadcast psum
            # tiles: borrows briefly shrink the ring to 2 but the steady state
            # keeps 3 slots so the QK->exp->QK round trip never paces the PE.
            ssp = ph2.enter_context(tc.tile_pool(name="ssp", bufs=3, space="PSUM"))
            cxp = ph2.enter_context(tc.tile_pool(name="cxp", bufs=1, space="PSUM"))
            ptp = ph2.enter_context(tc.tile_pool(name="ptp", bufs=6))
            mss = ph2.enter_context(tc.tile_pool(name="mss", bufs=8))

            def emit_oproj_one(c, j):
                # j in 0..7: one [128 q, 512 hid] block of chunk c's o_proj
                sq, nz = j // 2, j % 2
                r0 = c * CH + sq * 128
                ops = ssp.tile([128, 2 * CH], f32, tag="ss", name=f"op_{c}_{j}")
                nc.tensor.matmul(
                    ops[:, 0:512],
                    ctx_sb[:, r0 : r0 + 128],
                    wo_sb[:, nz * 512 : (nz + 1) * 512],
                    start=True,
                    stop=True,
                )
                ob = mss.tile([128, 512], f16, tag="ob", name=f"ob_{c}_{j}")
                # psum->sbuf fp16 staging, alternating engine
                if j % 2 == 0:
                    nc.vector.tensor_copy(ob, ops[:, 0:512])
                else:
                    nc.scalar.activation(ob, ops[:, 0:512], Ident)
                nc.sync.dma_start(
                    out[r0 : r0 + 128, nz * 512 : (nz + 1) * 512], ob
                )

            cx = [None, None]
            pts = [None] * SK

            def emit_qk_exp(c, i):
                cs0 = c * CH
                ksl = slice(i * 128, (i + 1) * 128)
                csl = slice(cs0, cs0 + CH)
                ss = ssp.tile([128, 2 * CH], f32, tag="ss", name=f"ss_{c}_{i}")
                for hh in range(2):
                    psl = slice(hh * 64, hh * 64 + 64)
                    nc.tensor.matmul(
                        ss[:, hh * CH : (hh + 1) * CH],
                        kp_sb[psl, ksl],
                        qT_sb[psl, csl],
                        start=True,
                        stop=True,
                    )
                pt = ptp.tile([128, 2 * CH], f16, tag="pt", name=f"pt_{c}_{i}")
                # 9/16 of tiles on ACT (table exp), 7/16 on DVE (Schraudolph):
                # balances engine busy incl. the other copies each one carries
                if (5 * i + c) % 16 < 9:
                    nc.scalar.activation(pt, ss, Exp, scale=0.125)
                else:
                    nc.vector.tensor_scalar(
                        pt[:, :].bitcast(i16), ss, EXP_A, EXP_B, Mult, Add
                    )
                pts[i] = pt

            def emit_pv(c, i):
                for hh in range(2):
                    nc.tensor.matmul(
                        cx[hh],
                        v1_sb[:, hh, i, :],
                        pts[i][:, hh * CH : (hh + 1) * CH],
                        start=(i == 0),
                        stop=(i == SK - 1),
                    )

            def emit_epilogue(c):
                cs0 = c * CH
                for hh in range(2):
                    hsl = slice(hh * 64, hh * 64 + 64)
                    # den -> Ln straight from psum while DVE stages the context
                    lnb = mss.tile([1, CH], f32, tag="lnb", name=f"lnb_{c}_{hh}")
                    nc.scalar.activation(lnb, cx[hh][64:65, :], Ln)
                    stage = mss.tile([64, CH], f16, tag="stage", name=f"stage_{c}_{hh}")
                    nc.vector.tensor_copy(stage, cx[hh][0:64, :])
                    rec = mss.tile([1, CH], f16, tag="rec", name=f"rec_{c}_{hh}")
                    # rec = 4096/den keeps fp16 in normal range; host undoes it
                    nc.scalar.activation(
                        rec, lnb, Exp, scale=-1.0, bias=ln4096_sb[:, :]
                    )
                    rb = ssp.tile([128, 2 * CH], f32, tag="ss", name=f"rb_{c}_{hh}")
                    nc.tensor.matmul(
                        rb[0:64, 0:512], ones_sb, rec, start=True, stop=True
                    )
                    nc.vector.tensor_mul(
                        ctx_sb[hsl, cs0 : cs0 + CH], stage, rb[0:64, 0:512]
                    )

            for c in range(NCH):
                cx[0] = cxp.tile([65, CH], f32, tag="cxA", name=f"cxA_{c}")
                cx[1] = cxp.tile([65, CH], f32, tag="cxB", name=f"cxB_{c}")
                for i in range(SK):
                    emit_qk_exp(c, i)
                    if i >= PVLAG:
                        emit_pv(c, i - PVLAG)
                    # one o_proj block of the previous chunk every 3rd iter
                    if c > 0 and i >= 6 and i < 30 and (i - 6) % 3 == 0:
                        emit_oproj_one(c - 1, (i - 6) // 3)
                for i in range(SK - PVLAG, SK):
                    emit_pv(c, i)
                emit_epilogue(c)
            for j in range(8):
                emit_oproj_one(NCH - 1, j)
    return nc


def _legalize_sync_waits(nc, max_waits=1):
    """Cap sync waits per instruction for this container's walrus build.

    The bundled walrus encodes a limited number of sync-wait commands per
    instruction ("Too many sync wait commands" codegen error), while Tile
    attaches one wait per logical processor where needed. An attached wait
    is equivalent to a standalone preceding wait on the same engine (that
    is exactly what raw-bass `wait_ge` emits: a pure-wait
    InstEventSemaphore), so hoist the excess waits onto EventSemaphore
    instructions inserted right before the offender.
    """
    from concourse import mybir

    n_fixed = 0
    for fn in nc.m.functions:
        for b in fn.blocks:
            insts = b.instructions
            idx = 0
            while idx < len(insts):
                inst = insts[idx]
                si = inst.sync_info
                waits = list(si.on_wait) if si and si.on_wait else []
                if len(waits) > max_waits:
                    updates = list(si.on_update) if si and si.on_update else []
                    pre, keep = waits[: -max_waits], waits[-max_waits:]
                    clones = []
                    for j, w in enumerate(pre):
                        clones.append(
                            mybir.InstEventSemaphore(
                                name=f"{inst.name}_sw{j}",
                                engine=inst.engine,
                                ins=[],
                                outs=[],
                                sync_info=mybir.SyncInfo(on_wait=[w], on_update=[]),
                            )
                        )
                    inst.sync_info = mybir.SyncInfo(on_wait=keep, on_update=updates)
                    for j, clone in enumerate(clones):
                        insts.insert(idx + j, clone)
                        try:
                            nc.inst_map[clone.name] = clone
                        except Exception:
                            pass
                    idx += len(clones)
                    n_fixed += 1
                idx += 1
    return n_fixed


MM_DT = "float16"


def get_nc(S=SEQ, mm_dt=MM_DT):
    key = S
    if key not in _NC_CACHE:
        nc = _build_nc(S)
        _legalize_sync_waits(nc)
        _NC_CACHE[key] = nc
    return _NC_CACHE[key]


def make_in_maps(h, cos, sin, wq, bq, wk, bk, wv, bv, wo):
    """Host-side shard prep. h [B,S,HID] -> per-core input dict."""
    f16 = np.float16
    h = np.asarray(h, dtype=np.float32)
    S = h.shape[1]
    hT = np.ascontiguousarray(h[0].T).astype(f16)  # [HID, S]
    cosT = np.ascontiguousarray(np.asarray(cos, np.float32).T).astype(f16)
    sinT = np.ascontiguousarray(np.asarray(sin, np.float32).T).astype(f16)
    wq = np.asarray(wq, dtype=np.float32)
    wk = np.asarray(wk, dtype=np.float32)
    wv = np.asarray(wv, dtype=np.float32)
    wo = np.asarray(wo, dtype=np.float32)
    bq = np.asarray(bq, dtype=np.float32)
    bk = np.asarray(bk, dtype=np.float32)
    in_maps = []
    for c in range(NCORES):
        fs = slice(c * FPC, (c + 1) * FPC)
        in_maps.append(
            {
                "hT": hT,
                "wqT": np.ascontiguousarray(wq[fs, :].T).astype(f16),
                "wkT": np.ascontiguousarray(wk[fs, :].T).astype(f16),
                "wvT": np.ascontiguousarray(wv[fs, :].T).astype(f16),
                "bqk": np.ascontiguousarray(
                    np.stack([bq[fs], bk[fs]], axis=1).astype(np.float32)
                ),
                "woT": np.ascontiguousarray(wo[:, fs].T).astype(f16),
                "cosT": cosT,
                "sinT": sinT,
            }
        )
    return in_maps


def kernel(h, mask, cos, sin, wq, bq, wk, bk, wv, bv, wo, bo, **_unused):
    # mask is all-ones per the problem spec; post-softmax where(mask==0) is a no-op.
    from concourse.bass_utils import run_bass_kernel_spmd

    h = np.asarray(h, dtype=np.float32)
    S = h.shape[1]
    nc = get_nc(S)
    in_maps = make_in_maps(h, cos, sin, wq, bq, wk, bk, wv, bv, wo)
    res = run_bass_kernel_spmd(nc, in_maps, core_ids=list(range(NCORES)))
    acc = np.zeros((S, HIDDEN), dtype=np.float64)
    for r in res.results:
        acc += r["out"].astype(np.float64)
    acc /= 4096.0
    bo_eff = np.asarray(bo, np.float64) + np.asarray(wo, np.float64) @ np.asarray(
        bv, np.float64
    )
    acc += bo_eff[None, :]
    return acc[None].astype(np.float32)
